# revision 15
# baseline (speedup 1.0000x reference)
"""Multi-head attention layer (B=4, L=48*48=2304, C=512, nh=8, dh=64) on 8 TRN2 cores.

Sharding: core c -> (b = c//2, query-half = c%2). Each core computes K/V for all
2304 tokens of its batch, Q for its 1152-token half, full attention for all 8
heads over its queries, and the output projection + residual for its tokens.
Outputs are disjoint row-slices of the final tensor -> no collectives needed.

Dataflow is fully "transposed" ([channels, tokens] layouts) so the PE never
needs a transpose:
  - host ships xT = x[b].T; Wqkv/Wo natural [c_in, c_out] layout serves as lhsT
  - qT/kT: psum[m_chunk, tok] = sum_cc W[cc, m].T @ xT[cc, tok]; bias is
    added during the DVE psum->SBUF evacuation (tensor_scalar_add with a
    per-partition bias column) so no ScalarE pass is needed
  - scoresT[key, qry] = kT_h.T @ qT_h   (1/sqrt(dh) folded into Wq host-side)
  - exp on ScalarE reading 3 psum banks per instruction (A/B head halves
    alternate so the next group's scores can reuse freed banks)
  - PV uses a 128-column stationary [v_h | ones] (even heads) / [ones | v_h]
    (odd heads): one FWL-eligible matmul per (head, key-chunk) yields both
    attn_outT rows AND 64 broadcast copies of sumexp in the same psum bank —
    no separate ones-matmul, and the 128-wide stationary keeps LDWEIGHTS in
    the background buffer so back-to-back PV matmuls stream at full rate
  - normalize: recip of the sumexp rows, rank-1 broadcast matmuls into the
    sumexp (garbage) halves of the OTHER head's bank, then two DVE multiplies
  - outT[c_out, tok] = Wo[cc, m].T @ attnT[cc, tok] + x.T + bo residual epilogue
"""

import ml_dtypes
import numpy as np

import concourse.bass as bass
import concourse.tile as tile
from concourse import bacc, mybir
from concourse.bass_utils import run_bass_kernel_spmd

F32 = mybir.dt.float32
BF16 = mybir.dt.bfloat16

B = 4
HW = 48
C = 512
L = HW * HW            # 2304 tokens
NH = 8                 # heads
DH = C // NH           # 64
NCORES = 8
LQ = L // 2            # 1152 queries per core
NCC = C // 128         # 4 contraction chunks of 128 channels
NKC = L // 128         # 18 key chunks of 128
QN = 384               # query tile (free dim of scores/PV matmuls)
NQT = LQ // QN         # 3 query tiles per core
NPAIR = NH // 2        # 4 head pairs
NG = NKC // 3          # 6 groups of 3 key-chunk-pairs (6 psum banks per group)


def build_ir(nc: bass.Bass) -> None:
    xT = nc.dram_tensor("xT", [C, L], BF16, kind="ExternalInput").ap()
    xr = nc.dram_tensor("xr", [C, LQ], F32, kind="ExternalInput").ap()
    wqkv = nc.dram_tensor("wqkv", [C, 3 * C], BF16, kind="ExternalInput").ap()
    wo = nc.dram_tensor("wo", [C, C], BF16, kind="ExternalInput").ap()
    bq = nc.dram_tensor("bq", [128, NCC], F32, kind="ExternalInput").ap()
    bk = nc.dram_tensor("bk", [128, NCC], F32, kind="ExternalInput").ap()
    bo = nc.dram_tensor("bo", [128, NCC], F32, kind="ExternalInput").ap()
    outT = nc.dram_tensor("outT", [C, LQ], F32, kind="ExternalOutput").ap()
    outT_r = outT.rearrange("(mc p) t -> p mc t", p=128)
    xT_r = xT.rearrange("(cc p) t -> p cc t", p=128)
    xr_r = xr.rearrange("(cc p) t -> p cc t", p=128)
    wqkv_r = wqkv.rearrange("(cc p) n -> p cc n", p=128)
    wo_r = wo.rearrange("(cc p) n -> p cc n", p=128)

    # this core's query-half columns of xT (sliced host-side per core%2)
    xq = nc.dram_tensor("xq", [C, LQ], BF16, kind="ExternalInput").ap()
    xq_r = xq.rearrange("(cc p) t -> p cc t", p=128)

    with tile.TileContext(nc) as tc:
        with (
            tc.tile_pool(name="const", bufs=1) as cpool,
            tc.tile_pool(name="persist", bufs=1) as pp,
            tc.tile_pool(name="work", bufs=4) as work,
            tc.tile_pool(name="psum", bufs=1, space="PSUM") as psum,
        ):
            # ---- constants
            bq_sb = cpool.tile([128, NCC], F32)
            nc.sync.dma_start(bq_sb[:], bq)
            bk_sb = cpool.tile([128, NCC], F32)
            nc.sync.dma_start(bk_sb[:], bk)
            bo_sb = cpool.tile([128, NCC], F32)
            nc.sync.dma_start(bo_sb[:], bo)
            ones64 = cpool.tile([128, 64], BF16)
            nc.vector.memset(ones64[:], 1.0)

            # ---- persistent intermediates
            xq_sb = pp.tile([128, NCC, LQ], BF16)
            qT_sb = pp.tile([128, NPAIR, LQ], BF16)
            kT_sb = pp.tile([128, NPAIR, L], BF16)
            # PV stationary: per (head, key-chunk) a 128-col [v | ones] or
            # [ones | v] block (even heads: v in cols 0-63; odd: cols 64-127)
            vx_sb = pp.tile([128, NH, NKC, 128], BF16)
            nc.vector.memset(vx_sb[:, 0:NH:2, :, DH:128], 1.0)
            nc.vector.memset(vx_sb[:, 1:NH:2, :, 0:DH], 1.0)
            attnT_sb = pp.tile([128, NCC, LQ], BF16)

            # ---- psum: 6 rotating score banks + per-head PV accumulators
            ps_s = psum.tile([128, 6, 512], F32)
            ps_a = psum.tile([128, 512], F32)
            ps_b = psum.tile([128, 512], F32)

            # ================= phase 1 + 2 interleaved =================
            # DMA priority: K then Q weight columns and the first half of
            # xT/xq feed the first kT/qT/score chains within ~6us; V columns
            # and the second xT half stream in behind them.
            wqkv_sb = pp.tile([128, NCC, 3 * C], BF16)
            xT_sb = pp.tile([128, NCC, L], BF16)
            for cc in range(NCC):
                nc.sync.dma_start(
                    wqkv_sb[:, cc, C : 2 * C], wqkv_r[:, cc, C : 2 * C]
                )
            for cc in range(NCC):
                nc.sync.dma_start(wqkv_sb[:, cc, 0:C], wqkv_r[:, cc, 0:C])
            for cc in range(NCC):
                nc.sync.dma_start(xT_sb[:, cc, 0:LQ], xT_r[:, cc, 0:LQ])
            for cc in range(NCC):
                nc.sync.dma_start(xq_sb[:, cc, :], xq_r[:, cc, :])
            for cc in range(NCC):
                nc.scalar.dma_start(
                    wqkv_sb[:, cc, 2 * C : 3 * C], wqkv_r[:, cc, 2 * C : 3 * C]
                )
            for cc in range(NCC):
                nc.scalar.dma_start(xT_sb[:, cc, LQ:L], xT_r[:, cc, LQ:L])

            bank_box = [0]

            def emit_v_chunk(tch):
                # V in [token, channel] layout: V[t,n] = xT[cc,t].T @ Wv + bv
                pb = ps_s[:, bank_box[0] % 6, 0:512]
                bank_box[0] += 1
                for cc in range(NCC):
                    nc.tensor.matmul(
                        pb,
                        xT_sb[:, cc, tch * 128 : (tch + 1) * 128],
                        wqkv_sb[:, cc, 2 * C : 3 * C],
                        start=(cc == 0),
                        stop=(cc == NCC - 1),
                    )
                # scatter per-head v columns into the [v|ones]/[ones|v] slots
                src = pb.rearrange("p (h d) -> p h d", d=DH)
                nc.vector.tensor_copy(
                    vx_sb[:, 0:NH:2, tch, 0:DH], src[:, 0:NH:2, :]
                )
                nc.vector.tensor_copy(
                    vx_sb[:, 1:NH:2, tch, DH:128], src[:, 1:NH:2, :]
                )

            def emit_kT_tile(m, g):
                # kT chunk m, key-token tile g (keys [g*QN, (g+1)*QN))
                t0 = g * QN
                pb = ps_s[:, bank_box[0] % 6, 0:QN]
                bank_box[0] += 1
                for cc in range(NCC):
                    nc.tensor.matmul(
                        pb,
                        wqkv_sb[:, cc, C + m * 128 : C + (m + 1) * 128],
                        xT_sb[:, cc, t0 : t0 + QN],
                        start=(cc == 0),
                        stop=(cc == NCC - 1),
                    )
                nc.vector.tensor_scalar_add(
                    kT_sb[:, m, t0 : t0 + QN], pb, bk_sb[:, m : m + 1]
                )

            def emit_qT_tile(m, qt):
                t0 = qt * QN
                pb = ps_s[:, bank_box[0] % 6, 0:QN]
                bank_box[0] += 1
                for cc in range(NCC):
                    nc.tensor.matmul(
                        pb,
                        wqkv_sb[:, cc, m * 128 : (m + 1) * 128],
                        xq_sb[:, cc, t0 : t0 + QN],
                        start=(cc == 0),
                        stop=(cc == NCC - 1),
                    )
                nc.vector.tensor_scalar_add(
                    qT_sb[:, m, t0 : t0 + QN], pb, bq_sb[:, m : m + 1]
                )

            def emit_norm_recip(p):
                # sumexp_A is broadcast across ps_a rows 64-127; sumexp_B
                # across ps_b rows 0-63. Recip both into a bf16 row pair;
                # the broadcast + multiply (emit_norm_apply) runs a cycle
                # later so this DVE chain never stalls the PE.
                recip_t = work.tile([128, QN], F32, tag="recip")
                # reciprocal_approx_fast misbehaves at base partition 64;
                # run it over [0:65] (rows 0-63 are junk recips of PV_A,
                # unused) so row 64 = 1/sumexp_A is computed correctly.
                nc.vector.reciprocal_approx_fast(
                    recip_t[0:65, :], ps_a[0:65, 0:QN]
                )
                nc.vector.reciprocal_approx_fast(
                    recip_t[0:1, :], ps_b[0:1, 0:QN]
                )
                recip_bf = work.tile([128, QN], BF16, tag="recipbf")
                nc.vector.tensor_copy(recip_bf[0:65, :], recip_t[0:65, :])
                return recip_bf

            def emit_norm_apply(p, q0, recip_bf):
                # bcast_A -> ps_b rows 0-63 (B's spent sumexp half)
                nc.tensor.matmul(
                    ps_b[0:64, 0:QN],
                    ones64[64:65, 0:64],
                    recip_bf[64:65, 0:QN],
                )
                # bcast_B -> ps_a rows 64-127 (A's spent sumexp half)
                nc.tensor.matmul(
                    ps_a[64:128, 0:QN],
                    ones64[0:1, 0:64],
                    recip_bf[0:1, 0:QN],
                )
                bcast_t = work.tile([128, QN], BF16, tag="bcast")
                nc.vector.tensor_copy(bcast_t[0:64, :], ps_b[0:64, 0:QN])
                nc.vector.tensor_copy(bcast_t[64:128, :], ps_a[64:128, 0:QN])
                nc.vector.tensor_mul(
                    attnT_sb[0:64, p, q0 : q0 + QN],
                    ps_a[0:64, 0:QN],
                    bcast_t[0:64, :],
                )
                nc.vector.tensor_mul(
                    attnT_sb[64:128, p, q0 : q0 + QN],
                    ps_b[64:128, 0:QN],
                    bcast_t[64:128, :],
                )

            def emit_scores(p, q0, g):
                # scoresT[key, qry]; heads A/B adjacent per chunk -> the PE
                # runs them concurrently as row-tiles (partitions 0-63/64-127).
                # A chunks -> banks 0-2, B chunks -> banks 3-5.
                for j in range(3):
                    kc = g * 3 + j
                    ks = slice(kc * 128, (kc + 1) * 128)
                    nc.tensor.matmul(
                        ps_s[:, j, 0:QN],
                        kT_sb[0:64, p, ks],
                        qT_sb[0:64, p, q0 : q0 + QN],
                    )
                    nc.tensor.matmul(
                        ps_s[:, 3 + j, 0:QN],
                        kT_sb[64:128, p, ks],
                        qT_sb[64:128, p, q0 : q0 + QN],
                    )

            def emit_exp(exp_t):
                # one ACTIVATE over all 6 banks (N=2304)
                nc.scalar.activation(
                    exp_t[:, :, :],
                    ps_s[:, 0:6, 0:QN],
                    mybir.ActivationFunctionType.Exp,
                )

            def emit_pv(p, g, exp_t):
                # One 128-col FWL matmul per (head, chunk): [v|ones] stationary
                # gives PV rows AND sumexp rows in a single accumulate chain.
                for j in range(3):
                    kc = g * 3 + j
                    st, sp = (kc == 0), (kc == NKC - 1)
                    nc.tensor.matmul(
                        ps_a[:, 0:QN],
                        vx_sb[:, 2 * p, kc, :],
                        exp_t[:, j, :],
                        start=st,
                        stop=sp,
                    )
                    nc.tensor.matmul(
                        ps_b[:, 0:QN],
                        vx_sb[:, 2 * p + 1, kc, :],
                        exp_t[:, 3 + j, :],
                        start=st,
                        stop=sp,
                    )

            def emit_out_tile(m, t0, dma_eng=None):
                pb = ps_s[:, bank_box[0] % 6, 0:QN]
                bank_box[0] += 1
                for cc in range(NCC):
                    nc.tensor.matmul(
                        pb,
                        wo_sb[:, cc, m * 128 : (m + 1) * 128],
                        attnT_sb[:, cc, t0 : t0 + QN],
                        start=(cc == 0),
                        stop=(cc == NCC - 1),
                    )
                ot = work.tile([128, QN], F32, tag="out")
                nc.vector.scalar_tensor_tensor(
                    ot[:],
                    pb,
                    bo_sb[:, m : m + 1],
                    xr_sb[:, m, t0 : t0 + QN],
                    op0=mybir.AluOpType.add,
                    op1=mybir.AluOpType.add,
                )
                (dma_eng or nc.sync).dma_start(
                    outT_r[:, m, t0 : t0 + QN], ot[:]
                )

            # output projection operands, DMA'd up front
            wo_sb = pp.tile([128, NCC, C], BF16)
            for cc in range(NCC):
                nc.scalar.dma_start(wo_sb[:, cc, :], wo_r[:, cc, :])
            xr_sb = pp.tile([128, NCC, LQ], F32)
            for cc in range(NCC):
                nc.scalar.dma_start(xr_sb[:, cc, :], xr_r[:, cc, :])

            # Projections are streamed just-in-time into the pipeline: pair
            # 0's projections feed its own first groups; pair p+1's ride
            # along pair p's last qtile.
            def prelude(p, qt, g):
                if p == 0 and qt == 0:
                    if g == 0:
                        emit_qT_tile(0, 0)
                    emit_kT_tile(0, g)
                    for j in range(3):
                        emit_v_chunk(3 * g + j)
                elif p == 0 and g == 0:
                    emit_qT_tile(0, qt)
                if qt == NQT - 1 and p < NPAIR - 1:
                    if g < NQT:
                        emit_qT_tile(p + 1, g)
                    emit_kT_tile(p + 1, g)

            # Software pipeline over all (pair, qtile, group) tiles: PV for
            # group t-1 is emitted after the scores of group t, so the PE has
            # independent work while the ACT exps group t's scores. The
            # normalize is split: reciprocals right after the PV chain stops,
            # the broadcast+multiply one group later (before the next qtile's
            # first PV) so its DVE chain never head-blocks the PE. Output
            # tiles for qtile qt are emitted as soon as pair 3's norm for qt
            # retires, overlapping phase 3 with the tail of the attention.
            groups = [
                (p, qt, g)
                for p in range(NPAIR)
                for qt in range(NQT)
                for g in range(NG)
            ]
            prev = None
            pending_norm = None
            pending_out = None
            for p, qt, g in groups:
                q0 = qt * QN
                prelude(p, qt, g)
                emit_scores(p, q0, g)
                if pending_norm is not None:
                    np_, nq0, nrecip = pending_norm
                    emit_norm_apply(np_, nq0, nrecip)
                    if np_ == NPAIR - 1:
                        pending_out = nq0
                    pending_norm = None
                if prev is not None:
                    pp_, pq0, pg, pexp = prev
                    emit_pv(pp_, pg, pexp)
                    if pg == NG - 1:
                        pending_norm = (pp_, pq0, emit_norm_recip(pp_))
                exp_t = work.tile([128, 6, QN], BF16, tag="expT")
                emit_exp(exp_t)
                if pending_out is not None:
                    for m in range(NCC):
                        emit_out_tile(m, pending_out)
                    pending_out = None
                prev = (p, q0, g, exp_t)
            pp_, pq0, pg, pexp = prev
            emit_pv(pp_, pg, pexp)
            rb = emit_norm_recip(pp_)
            emit_norm_apply(pp_, pq0, rb)

            # ================= phase 3 tail: last qtile's output =================
            for m in range(NCC):
                emit_out_tile(m, pq0, nc.scalar if m % 2 else nc.sync)


_compiled = None


def _get_compiled():
    global _compiled
    if _compiled is None:
        nc = bacc.Bacc(
            "TRN2", target_bir_lowering=False, debug=False, num_devices=NCORES
        )
        build_ir(nc)
        nc.compile()
        _compiled = nc
    return _compiled


def make_in_maps(x, Wqkv, bqkv, Wo, bo):
    x = np.asarray(x, np.float32)
    Wqkv = np.asarray(Wqkv, np.float32)
    bqkv = np.asarray(bqkv, np.float32)
    Wo = np.asarray(Wo, np.float32)
    bo = np.asarray(bo, np.float32)

    BF = ml_dtypes.bfloat16
    wqkv_mod = Wqkv.copy()
    wqkv_mod[:, :C] *= 1.0 / np.sqrt(DH)  # fold attention scale into Wq
    wqkv_mod = np.ascontiguousarray(wqkv_mod.astype(BF))
    bq_h = np.ascontiguousarray((bqkv[:C] / np.sqrt(DH)).reshape(NCC, 128).T)
    bk_h = np.ascontiguousarray(bqkv[C : 2 * C].reshape(NCC, 128).T)
    bo_eff = bo + bqkv[2 * C :] @ Wo
    bo_h = np.ascontiguousarray(bo_eff.reshape(NCC, 128).T)
    wo_c = np.ascontiguousarray(Wo.astype(BF))

    in_maps = []
    for c in range(NCORES):
        b, half = c // 2, c % 2
        xb = x[b].reshape(L, C)
        xTb = np.ascontiguousarray(xb.T)
        xTb_bf = xTb.astype(BF)
        sl = slice(half * LQ, (half + 1) * LQ)
        in_maps.append(
            {
                "xT": np.ascontiguousarray(xTb_bf),
                "xq": np.ascontiguousarray(xTb_bf[:, sl]),
                "xr": np.ascontiguousarray(xTb[:, sl]),
                "wqkv": wqkv_mod,
                "wo": wo_c,
                "bq": bq_h,
                "bk": bk_h,
                "bo": bo_h,
            }
        )
    return in_maps


def assemble_output(results):
    out = np.empty((B, L, C), np.float32)
    for c in range(NCORES):
        b, half = c // 2, c % 2
        out[b, half * LQ : (half + 1) * LQ, :] = results[c]["outT"].T
    return out.reshape(B, HW, HW, C)


def kernel(x, Wqkv, bqkv, Wo, bo):
    nc = _get_compiled()
    in_maps = make_in_maps(x, Wqkv, bqkv, Wo, bo)
    res = run_bass_kernel_spmd(nc, in_maps, list(range(NCORES)))
    return assemble_output(res.results)
